# revision 1
# baseline (speedup 1.0000x reference)
import numpy as np

B = 8
SEQ = 4096
D = 1024
N_BASE = 10000.0
N_CORES = 8
SPC = SEQ // N_CORES  # seq rows per core
JT = SPC // 128       # 128-row chunks per core
G_DEFAULT = 4         # batches per DMA group (descriptor size = G*16KB)

_CACHE = {}


def _compute_pe() -> np.ndarray:
    """Mirror of the reference _pos_encoding (default jax backend, f32)."""
    import jax
    import jax.numpy as jnp

    pos = jnp.arange(SEQ, dtype=jnp.float32)[:, None]
    i = jnp.arange(D // 2, dtype=jnp.float32)
    denom = jnp.power(jnp.float32(N_BASE), 2.0 * i / jnp.float32(D))
    ang = pos / denom
    pe = jnp.stack([jnp.sin(ang), jnp.cos(ang)], axis=-1).reshape(SEQ, D)
    return np.asarray(jax.device_get(pe), dtype=np.float32)


def _repack(x, c, G):
    xs = np.ascontiguousarray(x[:, c * SPC : (c + 1) * SPC, :])
    NG = B // G
    return np.ascontiguousarray(
        xs.reshape(NG, G, 128, JT, D).transpose(0, 2, 1, 3, 4)
    ).reshape(B * SPC, D)


def _unpack(y, G):
    NG = B // G
    return np.ascontiguousarray(
        y.reshape(NG, 128, G, JT, D).transpose(0, 2, 1, 3, 4)
    ).reshape(B, SPC, D)


def _build_program(G, pe_mid=False):
    import concourse.bacc as bacc
    import concourse.mybir as mybir
    import concourse.tile as tile

    NG = B // G
    K = G * JT
    nc = bacc.Bacc("TRN2")
    f32 = mybir.dt.float32
    x_in = nc.declare_dram_parameter("x", [B * SPC, D], f32, isOutput=False)
    pe_in = nc.declare_dram_parameter("pe", [SPC, D], f32, isOutput=False)
    y_out = nc.declare_dram_parameter("y", [B * SPC, D], f32, isOutput=True)

    with tile.TileContext(nc) as tc:
        with (
            tc.tile_pool(name="pe_pool", bufs=1) as pe_pool,
            tc.tile_pool(name="x_pool", bufs=NG) as x_pool,
        ):
            pe_t = pe_pool.tile([128, JT, D], f32)
            pe_ap = pe_in.rearrange("(p u) d -> p u d", u=JT)
            if not pe_mid:
                nc.sync.dma_start(out=pe_t[:], in_=pe_ap)
            xts = []
            for g in range(NG):
                xs = x_in[g * 128 * K : (g + 1) * 128 * K, :].rearrange(
                    "(p k) d -> p k d", k=K
                )
                xt = x_pool.tile([128, K, D], f32)
                nc.sync.dma_start(out=xt[:], in_=xs)
                xts.append(xt)
                if pe_mid and g == 0:
                    nc.sync.dma_start(out=pe_t[:], in_=pe_ap)
            for g in range(NG):
                for bb in range(G):
                    sl = xts[g][:, bb * JT : (bb + 1) * JT, :]
                    nc.vector.tensor_add(sl, sl, pe_t[:])
                ys = y_out[g * 128 * K : (g + 1) * 128 * K, :].rearrange(
                    "(p k) d -> p k d", k=K
                )
                nc.scalar.dma_start(out=ys, in_=xts[g][:])
    if not nc.is_finalized():
        nc.finalize()
    return nc


def _get_state(G=G_DEFAULT):
    if G not in _CACHE:
        _CACHE[G] = _build_program(G)
    if "pe" not in _CACHE:
        _CACHE["pe"] = _compute_pe()
    return _CACHE[G], _CACHE["pe"]


def kernel(x, seq_len=None, **_):
    from concourse.bass_utils import run_bass_kernel_spmd

    x = np.asarray(x, dtype=np.float32)
    assert x.shape == (B, SEQ, D)
    if seq_len is not None:
        assert int(np.asarray(seq_len)) == SEQ

    G = G_DEFAULT
    nc, pe = _get_state(G)
    in_maps = []
    for c in range(N_CORES):
        pes = np.ascontiguousarray(pe[c * SPC : (c + 1) * SPC, :])
        in_maps.append({"x": _repack(x, c, G), "pe": pes})

    res = run_bass_kernel_spmd(nc, in_maps, list(range(N_CORES))).results

    out = np.empty((B, SEQ, D), dtype=np.float32)
    for c in range(N_CORES):
        out[:, c * SPC : (c + 1) * SPC, :] = _unpack(res[c]["y"], G)
    return out



# revision 2
# speedup vs baseline: 1.0229x; 1.0229x over previous
import numpy as np

B = 8
SEQ = 4096
D = 1024
N_BASE = 10000.0
N_CORES = 8
SPC = SEQ // N_CORES  # seq rows per core (512)
JT = SPC // 128       # seq rows per partition per core (4)
KT = 2                # seq rows per partition per tile -> tile = 128*KT*D*4 bytes
NH = JT // KT         # tiles per batch
NT = B * NH           # tiles per core

_CACHE = {}


def _compute_pe() -> np.ndarray:
    """Mirror of the reference _pos_encoding (default jax backend, f32)."""
    import jax
    import jax.numpy as jnp

    pos = jnp.arange(SEQ, dtype=jnp.float32)[:, None]
    i = jnp.arange(D // 2, dtype=jnp.float32)
    denom = jnp.power(jnp.float32(N_BASE), 2.0 * i / jnp.float32(D))
    ang = pos / denom
    pe = jnp.stack([jnp.sin(ang), jnp.cos(ang)], axis=-1).reshape(SEQ, D)
    return np.asarray(jax.device_get(pe), dtype=np.float32)


def _repack(x, c):
    """Per-core x slice -> tile-major layout.

    Tile t = (b, h); tile row index = p*KT + j; holds x[b, c*SPC + p*JT + h*KT + j, :].
    """
    xs = np.ascontiguousarray(x[:, c * SPC : (c + 1) * SPC, :])
    return np.ascontiguousarray(
        xs.reshape(B, 128, NH, KT, D).transpose(0, 2, 1, 3, 4)
    ).reshape(NT * 128 * KT, D)


def _unpack(y):
    return np.ascontiguousarray(
        y.reshape(B, NH, 128, KT, D).transpose(0, 2, 1, 3, 4)
    ).reshape(B, SPC, D)


def _build_program():
    import concourse.bacc as bacc
    import concourse.mybir as mybir
    import concourse.tile as tile

    nc = bacc.Bacc("TRN2")
    f32 = mybir.dt.float32
    x_in = nc.declare_dram_parameter("x", [B * SPC, D], f32, isOutput=False)
    pe_in = nc.declare_dram_parameter("pe", [SPC, D], f32, isOutput=False)
    y_out = nc.declare_dram_parameter("y", [B * SPC, D], f32, isOutput=True)

    with tile.TileContext(nc) as tc:
        with (
            tc.tile_pool(name="pe_pool", bufs=1) as pe_pool,
            tc.tile_pool(name="x_pool", bufs=NT) as x_pool,
        ):
            pe_t = pe_pool.tile([128, JT, D], f32)
            pe_ap = pe_in.rearrange("(p u) d -> p u d", u=JT)
            # pe rides the (initially idle) store ring so x loads start at once
            nc.scalar.dma_start(out=pe_t[:], in_=pe_ap)
            xts = []
            for t in range(NT):
                xs = x_in[t * 128 * KT : (t + 1) * 128 * KT, :].rearrange(
                    "(p k) d -> p k d", k=KT
                )
                xt = x_pool.tile([128, KT, D], f32)
                nc.sync.dma_start(out=xt[:], in_=xs)
                xts.append(xt)
            for t in range(NT):
                h = t % NH
                sl = xts[t][:]
                nc.vector.tensor_add(sl, sl, pe_t[:, h * KT : (h + 1) * KT, :])
                ys = y_out[t * 128 * KT : (t + 1) * 128 * KT, :].rearrange(
                    "(p k) d -> p k d", k=KT
                )
                nc.scalar.dma_start(out=ys, in_=xts[t][:])
    if not nc.is_finalized():
        nc.finalize()
    return nc


def _get_state():
    if "nc" not in _CACHE:
        _CACHE["nc"] = _build_program()
    if "pe" not in _CACHE:
        _CACHE["pe"] = _compute_pe()
    return _CACHE["nc"], _CACHE["pe"]


def _make_in_maps(x):
    _, pe = _get_state()
    in_maps = []
    for c in range(N_CORES):
        pes = np.ascontiguousarray(pe[c * SPC : (c + 1) * SPC, :])
        in_maps.append({"x": _repack(x, c), "pe": pes})
    return in_maps


def kernel(x, seq_len=None, **_):
    from concourse.bass_utils import run_bass_kernel_spmd

    x = np.asarray(x, dtype=np.float32)
    assert x.shape == (B, SEQ, D)
    if seq_len is not None:
        assert int(np.asarray(seq_len)) == SEQ

    nc, _ = _get_state()
    in_maps = _make_in_maps(x)

    res = run_bass_kernel_spmd(nc, in_maps, list(range(N_CORES))).results

    out = np.empty((B, SEQ, D), dtype=np.float32)
    for c in range(N_CORES):
        out[:, c * SPC : (c + 1) * SPC, :] = _unpack(res[c]["y"])
    return out


# revision 3
# speedup vs baseline: 1.0268x; 1.0038x over previous
import numpy as np

B = 8
SEQ = 4096
D = 1024
N_BASE = 10000.0
N_CORES = 8
SPC = SEQ // N_CORES  # seq rows per core (512)
JT = SPC // 128       # seq rows per partition per core (4)
KT = 2                # rows/partition for the fine tiles (1 MiB)
NH = JT // KT         # fine tiles per batch
N_FINE = 4            # batches 0..N_FINE-1 use fine tiles; the rest full-batch tiles

_CACHE = {}


def _compute_pe() -> np.ndarray:
    """Mirror of the reference _pos_encoding (default jax backend, f32)."""
    import jax
    import jax.numpy as jnp

    pos = jnp.arange(SEQ, dtype=jnp.float32)[:, None]
    i = jnp.arange(D // 2, dtype=jnp.float32)
    denom = jnp.power(jnp.float32(N_BASE), 2.0 * i / jnp.float32(D))
    ang = pos / denom
    pe = jnp.stack([jnp.sin(ang), jnp.cos(ang)], axis=-1).reshape(SEQ, D)
    return np.asarray(jax.device_get(pe), dtype=np.float32)


def _tiles_spec():
    """(row_offset/128, rows_per_partition, pe_row_offset) per tile."""
    spec = []
    off = 0
    for b in range(B):
        if b < N_FINE:
            for h in range(NH):
                spec.append((off, KT, h * KT))
                off += KT
        else:
            spec.append((off, JT, 0))
            off += JT
    return spec


def _repack(x, c):
    """Per-core x slice -> tile-major DRAM layout matching _tiles_spec."""
    xs = np.ascontiguousarray(x[:, c * SPC : (c + 1) * SPC, :])
    blocks = []
    for b in range(B):
        xb = xs[b].reshape(128, NH, KT, D)  # seq = p*JT + h*KT + j
        if b < N_FINE:
            blocks.append(
                np.ascontiguousarray(xb.transpose(1, 0, 2, 3)).reshape(-1, D)
            )
        else:
            blocks.append(np.ascontiguousarray(xb).reshape(-1, D))
    return np.ascontiguousarray(np.concatenate(blocks, axis=0))


def _unpack(y):
    out = np.empty((B, SPC, D), dtype=np.float32)
    off = 0
    for b in range(B):
        blk = y[off * 128 : (off + JT) * 128]
        if b < N_FINE:
            out[b] = blk.reshape(NH, 128, KT, D).transpose(1, 0, 2, 3).reshape(
                SPC, D
            )
        else:
            out[b] = blk.reshape(SPC, D)
        off += JT
    return out


def _build_program():
    import concourse.bacc as bacc
    import concourse.mybir as mybir
    import concourse.tile as tile

    nc = bacc.Bacc("TRN2")
    f32 = mybir.dt.float32
    x_in = nc.declare_dram_parameter("x", [B * SPC, D], f32, isOutput=False)
    pe_in = nc.declare_dram_parameter("pe", [SPC, D], f32, isOutput=False)
    y_out = nc.declare_dram_parameter("y", [B * SPC, D], f32, isOutput=True)

    spec = _tiles_spec()
    with tile.TileContext(nc) as tc:
        with (
            tc.tile_pool(name="pe_pool", bufs=1) as pe_pool,
            tc.tile_pool(name="x_pool", bufs=1) as x_pool,
        ):
            pe_t = pe_pool.tile([128, JT, D], f32)
            # pe rides the (initially idle) store ring so x loads start at once
            nc.scalar.dma_start(
                out=pe_t[:], in_=pe_in.rearrange("(p u) d -> p u d", u=JT)
            )
            xts = []
            for i, (o, k, _po) in enumerate(spec):
                xs = x_in[o * 128 : (o + k) * 128, :].rearrange(
                    "(p k) d -> p k d", k=k
                )
                xt = x_pool.tile([128, k, D], f32, tag=f"t{i}")
                nc.sync.dma_start(out=xt[:], in_=xs)
                xts.append(xt)
            for i, (o, k, po) in enumerate(spec):
                sl = xts[i][:]
                nc.vector.tensor_add(sl, sl, pe_t[:, po : po + k, :])
                ys = y_out[o * 128 : (o + k) * 128, :].rearrange(
                    "(p k) d -> p k d", k=k
                )
                nc.scalar.dma_start(out=ys, in_=xts[i][:])
    if not nc.is_finalized():
        nc.finalize()
    return nc


def _get_state():
    if "nc" not in _CACHE:
        _CACHE["nc"] = _build_program()
    if "pe" not in _CACHE:
        _CACHE["pe"] = _compute_pe()
    return _CACHE["nc"], _CACHE["pe"]


def _make_in_maps(x):
    _, pe = _get_state()
    in_maps = []
    for c in range(N_CORES):
        pes = np.ascontiguousarray(pe[c * SPC : (c + 1) * SPC, :])
        in_maps.append({"x": _repack(x, c), "pe": pes})
    return in_maps


def kernel(x, seq_len=None, **_):
    from concourse.bass_utils import run_bass_kernel_spmd

    x = np.asarray(x, dtype=np.float32)
    assert x.shape == (B, SEQ, D)
    if seq_len is not None:
        assert int(np.asarray(seq_len)) == SEQ

    nc, _ = _get_state()
    in_maps = _make_in_maps(x)

    res = run_bass_kernel_spmd(nc, in_maps, list(range(N_CORES))).results

    out = np.empty((B, SEQ, D), dtype=np.float32)
    for c in range(N_CORES):
        out[:, c * SPC : (c + 1) * SPC, :] = _unpack(res[c]["y"])
    return out
